# revision 3
# baseline (speedup 1.0000x reference)
"""Modulated Conv2D (StyleGAN2-style) Trainium2 Bass kernel.

Problem shapes (hardcoded):
  x: [16, 256, 64, 64] f32    y: [16, 512] f32
  weights: [256, 256, 3, 3]   bias: [256]
  style_w: [256, 512]         style_b: [256]
  out: [16, 256, 64, 64] f32

Math identity used: instead of materializing per-sample modulated weights,
  out[b,o] = (1/wstd[b,o]) * conv(x[b] * style[b,:], w)[o] + bias[o]
  wstd[b,o] = sqrt(sum_i W2[o,i] * style[b,i]^2 + eps),  W2[o,i] = sum_kk w[o,i,kk]^2
so the conv weights are batch-independent (shared across samples/cores).

Sharding: data-parallel over batch, 2 samples per core across 8 cores.
Conv computed as 9 shifted matmuls (per kernel tap) accumulating in PSUM,
in bf16 with f32 accumulation.
"""

import numpy as np

import concourse.bass as bass
import concourse.tile as tile
from concourse import bacc, mybir
from concourse import bass_utils
from concourse.masks import make_identity

EPS = 1e-8
P = 128
B_LOC = 2          # samples per core
CIN, COUT = 256, 256
NI, NO = CIN // P, COUT // P   # 2, 2
S = 512
H = W = 64
KK = 9             # 3x3 taps
HP, WP = H + 2, W + 2  # zero-padded image
N_CORES = 8

F32 = mybir.dt.float32
BF16 = mybir.dt.bfloat16


def build_conv2dmod(nc):
    x = nc.dram_tensor("x", [B_LOC, CIN, H, W], F32, kind="ExternalInput")
    y = nc.dram_tensor("y", [B_LOC, S], F32, kind="ExternalInput")
    weights = nc.dram_tensor("weights", [COUT, CIN, 3, 3], F32, kind="ExternalInput")
    bias = nc.dram_tensor("bias", [COUT], F32, kind="ExternalInput")
    style_w = nc.dram_tensor("style_w", [CIN, S], F32, kind="ExternalInput")
    style_b = nc.dram_tensor("style_b", [CIN], F32, kind="ExternalInput")
    out = nc.dram_tensor("out", [B_LOC, COUT, H, W], F32, kind="ExternalOutput")

    with tile.TileContext(nc) as tc:
        with (
            tc.tile_pool(name="consts", bufs=1) as consts,
            tc.tile_pool(name="temps", bufs=1) as temps,
            tc.tile_pool(name="xin_pool", bufs=3) as xin_pool,
            tc.tile_pool(name="xs_pool", bufs=1) as xs_pool,
            tc.tile_pool(name="out_pool", bufs=3) as out_pool,
            tc.tile_pool(name="psum", bufs=2, space="PSUM") as psum,
        ):
            # ---------- constant loads ----------
            identity = consts.tile([P, P], BF16)
            make_identity(nc, identity)

            w_nat = []      # [o_part, i, kk] f32, natural layout per o-tile
            for ot in range(NO):
                t = consts.tile([P, CIN, KK], F32, name=f"w_nat{ot}", tag=f"w_nat{ot}")
                nc.sync.dma_start(
                    t[:],
                    weights.ap()[ot * P:(ot + 1) * P].rearrange("o i kh kw -> o i (kh kw)"),
                )
                w_nat.append(t)

            sw_nat = []     # style_w natural [i_part, s]
            for it in range(NI):
                t = consts.tile([P, S], F32, name=f"sw_nat{it}", tag=f"sw_nat{it}")
                nc.sync.dma_start(t[:], style_w.ap()[it * P:(it + 1) * P, :])
                sw_nat.append(t)

            # y broadcast to all partitions: [128, B_LOC, S]
            y_bcast = consts.tile([P, B_LOC, S], F32)
            nc.sync.dma_start(y_bcast[:], y.ap()[None].to_broadcast((P, B_LOC, S)))

            # bias / style_b striped to [128, n_tiles]
            bias_col = consts.tile([P, NO], F32)
            nc.sync.dma_start(bias_col[:], bias.ap().rearrange("(oo oi) -> oi oo", oi=P))
            style_b_col = consts.tile([P, NI], F32)
            nc.sync.dma_start(style_b_col[:], style_b.ap().rearrange("(io ii) -> ii io", ii=P))

            # ---------- style = y @ style_w.T + style_b, per i-tile ----------
            # style_col[it][:, b] = style for channels it*128..it*128+127 of sample b
            style_col = []
            style2 = []
            for it in range(NI):
                tmp = temps.tile([P, B_LOC, S], F32, name=f"style_tmp{it}", tag="style_tmp")
                nc.vector.tensor_mul(
                    tmp[:], y_bcast[:], sw_nat[it][:, None, :].to_broadcast((P, B_LOC, S))
                )
                sc = consts.tile([P, B_LOC], F32, name=f"style_col{it}", tag=f"style_col{it}")
                nc.vector.reduce_sum(sc[:], tmp[:], axis=mybir.AxisListType.X)
                nc.vector.tensor_scalar_add(sc[:], sc[:], style_b_col[:, it:it + 1])
                s2 = consts.tile([P, B_LOC], F32, name=f"style2{it}", tag=f"style2{it}")
                nc.vector.tensor_mul(s2[:], sc[:], sc[:])
                style_col.append(sc)
                style2.append(s2)

            # ---------- weights: cast to bf16, transpose to [i, o, kk] ----------
            w_nat_bf = []
            for ot in range(NO):
                t = consts.tile([P, CIN, KK], BF16, name=f"w_nat_bf{ot}", tag=f"w_nat_bf{ot}")
                nc.vector.tensor_copy(t[:], w_nat[ot][:])
                w_nat_bf.append(t)

            w_bf = []       # [i_part, o, kk] bf16 — lhsT layout
            for it in range(NI):
                t = consts.tile([P, COUT, KK], BF16, name=f"w_bf{it}", tag=f"w_bf{it}")
                w_bf.append(t)

            tcount = 0
            for ot in range(NO):
                for it in range(NI):
                    for kk in range(KK):
                        pt = psum.tile([P, P], BF16, name=f"tp{ot}_{it}_{kk}",
                                       tag=f"ch{tcount % 4}")
                        tcount += 1
                        nc.tensor.transpose(
                            pt[:], w_nat_bf[ot][:, it * P:(it + 1) * P, kk], identity[:]
                        )
                        nc.vector.tensor_copy(w_bf[it][:, ot * P:(ot + 1) * P, kk], pt[:])

            # ---------- W2T[i, o] = sum_kk w[o,i,kk]^2  (i on partitions) ----------
            w2t = []
            for it in range(NI):
                sq = temps.tile([P, COUT, KK], F32, name=f"sq{it}", tag="sq")
                nc.vector.tensor_mul(sq[:], w_bf[it][:], w_bf[it][:])
                t = consts.tile([P, COUT], F32, name=f"w2t{it}", tag=f"w2t{it}")
                nc.vector.reduce_sum(t[:], sq[:], axis=mybir.AxisListType.X)
                w2t.append(t)

            # ---------- demod: winv[o, b] = 1/sqrt(W2T.T @ style2 + eps) ----------
            eps_col = consts.tile([P, 1], F32)
            nc.vector.memset(eps_col[:], EPS)
            winv = []
            for ot in range(NO):
                ps = psum.tile([P, B_LOC], F32, name=f"sig{ot}", tag=f"ch{ot}")
                for it in range(NI):
                    nc.tensor.matmul(
                        ps[:], w2t[it][:, ot * P:(ot + 1) * P], style2[it][:],
                        start=(it == 0), stop=(it == NI - 1),
                    )
                wstd = consts.tile([P, B_LOC], F32, name=f"wstd{ot}", tag=f"wstd{ot}")
                nc.scalar.activation(wstd[:], ps[:], mybir.ActivationFunctionType.Sqrt,
                                     bias=eps_col[:])
                wi = consts.tile([P, B_LOC], F32, name=f"winv{ot}", tag=f"winv{ot}")
                nc.vector.reciprocal(wi[:], wstd[:])
                winv.append(wi)

            # ---------- input stage: load x, scale by style, cast bf16, pad ----------
            xs = {}         # (b, it) -> [128, 66, 66] bf16 zero-padded scaled input
            for b in range(B_LOC):
                for it in range(NI):
                    xp = xs_pool.tile([P, HP, WP], BF16, name=f"xs{b}_{it}", tag=f"xs{b}_{it}")
                    # zero the 1-px border
                    nc.gpsimd.memset(xp[:, 0, :], 0.0)
                    nc.gpsimd.memset(xp[:, HP - 1, :], 0.0)
                    nc.gpsimd.memset(xp[:, 1:HP - 1, 0], 0.0)
                    nc.gpsimd.memset(xp[:, 1:HP - 1, WP - 1], 0.0)
                    xin = xin_pool.tile([P, H, W], F32, name=f"xin{b}_{it}", tag="xin")
                    nc.sync.dma_start(xin[:], x.ap()[b, it * P:(it + 1) * P])
                    # interior = x * style (per-channel), f32 -> bf16 on ScalarE
                    nc.scalar.activation(
                        xp[:, 1:H + 1, 1:W + 1], xin[:],
                        mybir.ActivationFunctionType.Copy,
                        scale=style_col[it][:, b:b + 1],
                    )
                    xs[(b, it)] = xp

            # ---------- main conv: 9 shifted matmuls, bf16, f32 PSUM accum ----------
            # loop (b, ot, half): 4 psum banks of [128, 8, 64] each (32 rows)
            for b in range(B_LOC):
                for ot in range(NO):
                    for half in range(2):
                        r0 = half * 32
                        oh = out_pool.tile([P, 32, W], F32, name=f"oh{b}{ot}{half}", tag="oh")
                        pcs = [
                            psum.tile([P, 8, W], F32, name=f"pc{b}{ot}{half}_{c}", tag=f"ch{c}")
                            for c in range(4)
                        ]
                        first, last = (0, 0), (NI - 1, KK - 1)
                        for it in range(NI):
                            for kk in range(KK):
                                dy, dx = kk // 3, kk % 3
                                lhsT = w_bf[it][:, ot * P:(ot + 1) * P, kk]
                                for c in range(4):
                                    rs = r0 + c * 8 + dy
                                    rhs = xs[(b, it)][:, rs:rs + 8, dx:dx + W]
                                    nc.tensor.matmul(
                                        pcs[c][:], lhsT, rhs,
                                        start=((it, kk) == first), stop=((it, kk) == last),
                                    )
                        for c in range(4):
                            nc.scalar.activation(
                                oh[:, c * 8:(c + 1) * 8, :], pcs[c][:],
                                mybir.ActivationFunctionType.Identity,
                                bias=bias_col[:, ot:ot + 1],
                                scale=winv[ot][:, b:b + 1],
                            )
                        nc.sync.dma_start(
                            out.ap()[b, ot * P:(ot + 1) * P, r0:r0 + 32, :], oh[:]
                        )
    return nc


_CACHED_NC = None


def _get_nc():
    global _CACHED_NC
    if _CACHED_NC is None:
        nc = bacc.Bacc("TRN2", target_bir_lowering=False, debug=False,
                       num_devices=N_CORES)
        build_conv2dmod(nc)
        nc.compile()
        _CACHED_NC = nc
    return _CACHED_NC


def kernel(x, y, weights, bias, style_w, style_b, _trace=False):
    x = np.ascontiguousarray(np.asarray(x, dtype=np.float32))
    y = np.ascontiguousarray(np.asarray(y, dtype=np.float32))
    weights = np.ascontiguousarray(np.asarray(weights, dtype=np.float32))
    bias = np.ascontiguousarray(np.asarray(bias, dtype=np.float32))
    style_w = np.ascontiguousarray(np.asarray(style_w, dtype=np.float32))
    style_b = np.ascontiguousarray(np.asarray(style_b, dtype=np.float32))

    nc = _get_nc()
    in_maps = [
        {
            "x": np.ascontiguousarray(x[c * B_LOC:(c + 1) * B_LOC]),
            "y": np.ascontiguousarray(y[c * B_LOC:(c + 1) * B_LOC]),
            "weights": weights,
            "bias": bias,
            "style_w": style_w,
            "style_b": style_b,
        }
        for c in range(N_CORES)
    ]
    res = bass_utils.run_bass_kernel_spmd(
        nc, in_maps, core_ids=list(range(N_CORES)), trace=_trace
    )
    out = np.concatenate([r["out"] for r in res.results], axis=0)
    if _trace:
        kernel.last_results = res
    return out
